# revision 1
# baseline (speedup 1.0000x reference)
"""Two-layer GATv2 (heads=1, edge_dim=1) on 8 Trainium2 NeuronCores.

Sharding: nodes are dealt round-robin by in-degree onto 8 cores (balances node
and edge counts and makes the per-block max-degree profile identical across
cores, so one SPMD program fits all 8). Destination-grouped edges stay local to
the owning shard; source features come from an AllGather'd full table via bulk
int16 dma_gather (two-table split covers 50k rows).

Per 128-node block (node = partition):
  u  = w (x) We~ + xr~              -- PE matmuls into PSUM (WeDiag/IdTile consts)
  v  = u + xl~[src]                 -- gathered rows folded in via PE I@g matmul
  m  = lrelu(v)                     -- ACT
  s  = sum_{d<P1} m - sum_{d>=P1} m -- DVE reduces; att sign-split + |att| scale
                                       are folded into the tables host-side
  alpha = exp(s)*mask / Z           -- no max-subtraction (|s| stays small)
  agg = sum_k alpha * v             -- DVE mult + strided reduce, then rank-1
                                       corrections remove the xr~/We~ parts
  h' = elu(agg/|att| + bias) + 1    -- the +1 is folded into layer-2 biases
Layer 2 identical with D=32 (table rows padded to 64 cols for the 256B gather
elem), then softplus + 1e-4. Host un-permutes rows (node deal) and columns
(att2 sign sort).
"""

import numpy as np

N, E, D_IN, DH, DO = 50000, 800000, 128, 64, 32
C = 8                      # cores
NL = N // C                # nodes per core (6250)
P = 128                    # partitions = nodes per block
NB = (NL + P - 1) // P     # blocks per core (49)
NLP = NB * P               # padded nodes per core (6272)
SPLIT = 32768              # int16 gather table split
VSMALL = 32                # K threshold for double-buffered PSUM v-tiles
DBG = False                # add layer-1 block-0 debug dumps
STAGE = 0                  # 0=full, 1=stop after AG1, 2=+one gather, 3=+one block


# ----------------------------------------------------------------------------
# host-side: weight folding and graph layout
# ----------------------------------------------------------------------------

def _fold(Wl, bl, Wr, br, We, att, bias, in_perm=None, h_offset=False):
    att = np.asarray(att, np.float64)
    pi = np.concatenate([np.nonzero(att >= 0)[0], np.nonzero(att < 0)[0]])
    p1 = int((att >= 0).sum())
    a = np.maximum(np.abs(att[pi]), 1e-30)
    Wl = np.asarray(Wl, np.float64)[pi] * a[:, None]
    Wr = np.asarray(Wr, np.float64)[pi] * a[:, None]
    bl = np.asarray(bl, np.float64)[pi] * a
    br = np.asarray(br, np.float64)[pi] * a
    We_ = np.asarray(We, np.float64)[pi, 0] * a
    if in_perm is not None:
        Wl = Wl[:, in_perm]
        Wr = Wr[:, in_perm]
    if h_offset:  # input arrives as h+1
        bl = bl - Wl.sum(1)
        br = br - Wr.sum(1)
    return dict(
        WlT=np.ascontiguousarray(Wl.T, np.float32),
        WrT=np.ascontiguousarray(Wr.T, np.float32),
        bl=bl.astype(np.float32)[:, None], br=br.astype(np.float32)[:, None],
        We=We_.astype(np.float32),
        inva=(1.0 / a).astype(np.float32),
        bias=np.asarray(bias, np.float64)[pi].astype(np.float32),
        pi=pi, p1=p1,
    )


def _prep(x, edge_index, edge_weight):
    src = np.asarray(edge_index[0], np.int64)
    dst = np.asarray(edge_index[1], np.int64)
    w = np.asarray(edge_weight, np.float32)

    deg = np.bincount(dst, minlength=N)
    wsum = np.bincount(dst, weights=w.astype(np.float64), minlength=N)
    loop_w = (wsum / np.maximum(deg, 1)).astype(np.float32)

    order = np.argsort(-deg, kind="stable")
    new_id = np.empty(N, np.int64)
    ranks = np.arange(N)
    new_id[order] = (ranks % C) * NL + ranks // C
    inv = np.empty(N, np.int64)
    inv[new_id] = np.arange(N)           # old id of each new id

    esrc = np.concatenate([new_id[src], np.arange(N)])
    edst = np.concatenate([new_id[dst], np.arange(N)])
    ew = np.concatenate([w, loop_w[inv]]).astype(np.float32)

    side = (esrc >= SPLIT).astype(np.int64)
    eord = np.argsort(edst * 2 + side, kind="stable")
    sdst, ssrc, sw, sside = edst[eord], esrc[eord], ew[eord], side[eord]

    nA = np.bincount(edst[side == 0], minlength=N)
    nB = np.bincount(edst[side == 1], minlength=N)

    grp = np.searchsorted(sdst, np.arange(N))
    pos = np.arange(E + N) - grp[sdst]
    posAB = np.where(sside == 0, pos, pos - nA[sdst])

    nblk = (np.arange(N) % NL) // P      # block of each new id
    KA = np.zeros(NB, np.int64)
    KB = np.zeros(NB, np.int64)
    np.maximum.at(KA, nblk, nA)
    np.maximum.at(KB, nblk, nB)
    KA = np.maximum(KA, 1)
    K = KA + KB
    assert int(K.max()) * DH * 4 <= 16384, f"KMAX {K.max()} overflows PSUM"
    koff = np.concatenate([[0], np.cumsum(K)])[:-1]
    totK = int(K.sum())
    KMAX = int(K.max())

    e_core = sdst // NL
    e_loc = sdst % NL
    e_blk = e_loc // P
    e_p = e_loc % P
    e_k = np.where(sside == 0, posAB, KA[e_blk] + posAB)

    w_arr = np.zeros((C, P, totK), np.float32)
    m01 = np.zeros((C, P, totK), np.float32)
    col = koff[e_blk] + e_k
    w_arr[e_core, e_p, col] = sw
    m01[e_core, e_p, col] = 1.0
    if NL % P:  # dummy partitions in last block: avoid Z=0
        m01[:, NL % P:, koff[-1]] = 1.0

    colsA = np.concatenate([[0], np.cumsum(KA * 8)]).astype(np.int64)
    colsB = np.concatenate([[0], np.cumsum(KB * 8)]).astype(np.int64)
    idxA = np.zeros((C, 128, int(colsA[-1])), np.int16)
    idxB = np.zeros((C, 128, max(int(colsB[-1]), 16)), np.int16)
    mA = sside == 0
    fA = e_k[mA] * P + e_p[mA]
    idxA[e_core[mA], fA % 16, colsA[e_blk[mA]] + fA // 16] = \
        ssrc[mA].astype(np.int16)
    mB = ~mA
    fB = (e_k[mB] - KA[e_blk[mB]]) * P + e_p[mB]
    idxB[e_core[mB], fB % 16, colsB[e_blk[mB]] + fB // 16] = \
        (ssrc[mB] - SPLIT).astype(np.int16)
    for rep in range(1, 8):
        idxA[:, 16 * rep:16 * rep + 16] = idxA[:, :16]
        idxB[:, 16 * rep:16 * rep + 16] = idxB[:, :16]

    x = np.asarray(x, np.float32)
    xT = np.zeros((C, D_IN, NLP), np.float32)
    perm = inv.reshape(C, NL)
    for c in range(C):
        xT[c, :, :NL] = x[perm[c]].T

    wT = np.zeros((C, KMAX, NLP), np.float32)
    wT[e_core, e_k, e_blk * P + e_p] = sw

    return dict(new_id=new_id, K=K, KA=KA, KB=KB, koff=koff, totK=totK,
                KMAX=KMAX, colsA=colsA, colsB=colsB, w_arr=w_arr, m01=m01,
                idxA=idxA, idxB=idxB, xT=xT, wT=wT)


def _consts(KMAX, We1, We2):
    WeDiag1 = np.zeros((KMAX, KMAX * DH), np.float32)
    IdTile1 = np.zeros((DH, KMAX * DH), np.float32)
    WeDiag2 = np.zeros((KMAX, KMAX * DH), np.float32)
    IdTile2 = np.zeros((DO, KMAX * DH), np.float32)
    for k in range(KMAX):
        WeDiag1[k, k * DH:(k + 1) * DH] = We1
        WeDiag2[k, k * DH:k * DH + DO] = We2
    for d in range(DH):
        IdTile1[d, d::DH] = 1.0
    for d in range(DO):
        IdTile2[d, d::DH] = 1.0
    return WeDiag1, IdTile1, WeDiag2, IdTile2


# ----------------------------------------------------------------------------
# device program
# ----------------------------------------------------------------------------

def _build(plan):
    import concourse.bacc as bacc
    import concourse.bass as bass
    import concourse.mybir as mybir
    import concourse.tile as tile
    from concourse.library_config import mlp
    from concourse.masks import make_identity

    f32 = mybir.dt.float32
    i16 = mybir.dt.int16
    Op = mybir.AluOpType
    Act = mybir.ActivationFunctionType

    K, KA, KB = plan["K"], plan["KA"], plan["KB"]
    koff, totK, KMAX = plan["koff"], plan["totK"], plan["KMAX"]
    colsA, colsB = plan["colsA"], plan["colsB"]
    P1, P2 = plan["p1"], plan["p2"]
    nA_cols = int(colsA[-1])
    nB_cols = max(int(colsB[-1]), 16)

    nc = bacc.Bacc("TRN2", debug=False)

    def din(name, shape, dt=f32):
        return nc.dram_tensor(name, shape, dt, kind="ExternalInput")

    xT_d = din("xT", [D_IN, NLP])
    idxA_d = din("idxA", [128, nA_cols], i16)
    idxB_d = din("idxB", [128, nB_cols], i16)
    w_d = din("w_arr", [P, totK])
    m01_d = din("m01", [P, totK])
    wT_d = din("wT", [KMAX, NLP])
    Wl1T_d, Wr1T_d = din("Wl1T", [D_IN, DH]), din("Wr1T", [D_IN, DH])
    bl1_d, br1_d = din("bl1", [DH, 1]), din("br1", [DH, 1])
    Wl2T_d, Wr2T_d = din("Wl2T", [DH, DO]), din("Wr2T", [DH, DO])
    bl2_d, br2_d = din("bl2", [DO, 1]), din("br2", [DO, 1])
    WeDiag1_d = din("WeDiag1", [KMAX, KMAX * DH])
    IdTile1_d = din("IdTile1", [DH, KMAX * DH])
    WeDiag2_d = din("WeDiag2", [KMAX, KMAX * DH])
    IdTile2_d = din("IdTile2", [DO, KMAX * DH])
    We1r_d, inva1_d, bias1_d = din("We1r", [1, DH]), din("inva1", [1, DH]), \
        din("bias1", [1, DH])
    We2r_d, inva2_d, bias2_d = din("We2r", [1, DO]), din("inva2", [1, DO]), \
        din("bias2", [1, DO])

    out_d = nc.dram_tensor("out", [NLP, DO], f32, kind="ExternalOutput")
    dbg = {}
    if DBG:
        KW = int(K[0]) * DH
        for nm, sh in [("dbg_g", [P, KW]), ("dbg_v", [P, KW]),
                       ("dbg_m", [P, KW]), ("dbg_s", [P, int(K[0])]),
                       ("dbg_e2", [P, int(K[0])]), ("dbg_al", [P, int(K[0])]),
                       ("dbg_agg", [P, DH]), ("dbg_t5", [P, DH]),
                       ("dbg_h", [P, DH])]:
            dbg[nm] = nc.dram_tensor(nm, sh, f32, kind="ExternalOutput")

    bounce1 = nc.dram_tensor("bounce1", [NL, DH], f32)
    table1 = nc.dram_tensor("table1", [N, DH], f32)
    bounce2 = nc.dram_tensor("bounce2", [NL, DH], f32)
    table2 = nc.dram_tensor("table2", [N, DH], f32)
    xrT_dram = nc.dram_tensor("xrT_dram", [DH, NLP], f32)
    xr2T_dram = nc.dram_tensor("xr2T_dram", [DO, NLP], f32)

    with tile.TileContext(nc) as tc:
      with tc.tile_pool(name="persist", bufs=1) as pp:
        ident = pp.tile([P, P], f32)
        make_identity(nc, ident[:])
        nc.gpsimd.load_library(mlp)

        idxA_t = pp.tile([128, nA_cols], i16)
        idxB_t = pp.tile([128, nB_cols], i16)
        w_t = pp.tile([P, totK], f32)
        m01_t = pp.tile([P, totK], f32)
        xr_nm = pp.tile([P, NB * DH], f32)
        h_t = pp.tile([P, NB * DH], f32)
        xr2_nm = pp.tile([P, NB * DH], f32)
        nc.vector.memset(xr2_nm[:], 0.0)
        WeDiag1_t = pp.tile([KMAX, KMAX * DH], f32)
        IdTile1_t = pp.tile([DH, KMAX * DH], f32)
        WeDiag2_t = pp.tile([KMAX, KMAX * DH], f32)
        IdTile2_t = pp.tile([DO, KMAX * DH], f32)
        We1r_t = pp.tile([P, DH], f32)
        inva1_t = pp.tile([P, DH], f32)
        bias1_t = pp.tile([P, DH], f32)
        We2r_t = pp.tile([P, DO], f32)
        inva2_t = pp.tile([P, DO], f32)
        bias2_t = pp.tile([P, DO], f32)
        Wl1T_t = pp.tile([D_IN, DH], f32, tag="Wl1T_t")
        Wr1T_t = pp.tile([D_IN, DH], f32, tag="Wr1T_t")
        bl1_t = pp.tile([DH, 1], f32, tag="bl1_t")
        br1_t = pp.tile([DH, 1], f32, tag="br1_t")
        Wl2T_t = pp.tile([DH, DO], f32, tag="Wl2T_t")
        Wr2T_t = pp.tile([DH, DO], f32, tag="Wr2T_t")
        bl2_t = pp.tile([DO, 1], f32, tag="bl2_t")
        br2_t = pp.tile([DO, 1], f32, tag="br2_t")

        for t, d in [(idxA_t, idxA_d), (idxB_t, idxB_d), (w_t, w_d),
                     (m01_t, m01_d),
                     (WeDiag1_t, WeDiag1_d), (IdTile1_t, IdTile1_d),
                     (WeDiag2_t, WeDiag2_d), (IdTile2_t, IdTile2_d),
                     (Wl1T_t, Wl1T_d), (Wr1T_t, Wr1T_d),
                     (bl1_t, bl1_d), (br1_t, br1_d),
                     (Wl2T_t, Wl2T_d), (Wr2T_t, Wr2T_d),
                     (bl2_t, bl2_d), (br2_t, br2_d)]:
            nc.sync.dma_start(t[:], d[:])
        for t, d, dd in [(We1r_t, We1r_d, DH), (inva1_t, inva1_d, DH),
                         (bias1_t, bias1_d, DH), (We2r_t, We2r_d, DO),
                         (inva2_t, inva2_d, DO), (bias2_t, bias2_d, DO)]:
            nc.sync.dma_start(t[:], d[:].to_broadcast([P, dd]))

        def bcast_inner(ap, n):
            return bass.AP(ap.tensor, ap.offset, [*ap.ap, [0, n]])

        # ------------------ dense phase (either layer) -------------------
        def dense(DOUT, rhsT_ap, WlT, blb, WrT, brb, xrT_dr, nm_r, bounce, pad):
            """Per block: xl = W~l-matmul + bias -> node-major -> bounce DRAM
            (zero-padded to DH cols if pad); xr~ written feature-major to
            xrT_dr DRAM and node-major into nm_r."""
            with (
                tc.tile_pool(name="dps", bufs=2, space="PSUM") as dps,
                tc.tile_pool(name="dsb", bufs=3) as dsb,
            ):
                for j in range(NB):
                    rhs = rhsT_ap[:, j * P:(j + 1) * P]
                    for which, (WT, bb) in enumerate(((WlT, blb), (WrT, brb))):
                        ps = dps.tile([DOUT, P], f32, tag="mm", space="PSUM")
                        nc.tensor.matmul(out=ps[:], lhsT=WT, rhs=rhs,
                                         start=True, stop=True)
                        ft = dsb.tile([DOUT, P], f32, tag="ft")
                        nc.scalar.activation(out=ft[:], in_=ps[:],
                                             func=Act.Identity, bias=bb)
                        if which == 1:
                            pass
                        ps2 = dps.tile([P, DOUT], f32, tag="tr", space="PSUM")
                        nc.tensor.transpose(out=ps2[:], in_=ft[:],
                                            identity=ident[:DOUT, :DOUT])
                        if which == 0:
                            st = dsb.tile([P, DH], f32, tag="st")
                            if pad:
                                nc.vector.memset(st[:], 0.0)
                            nc.vector.tensor_copy(out=st[:, :DOUT], in_=ps2[:])
                            lo, hi = j * P, min((j + 1) * P, NL)
                            if hi > lo:
                                nc.sync.dma_start(out=bounce[lo:hi, :],
                                                  in_=st[:hi - lo, :])
                        else:
                            stride = DH if pad else DOUT
                            nc.vector.tensor_copy(
                                out=nm_r[:, j * stride:j * stride + DOUT],
                                in_=ps2[:])

        # ------------------ edge phase (either layer) --------------------
        def edge(layer):
            if layer == 1:
                D, p1, table, WeD_t, DIN2 = DH, P1, table1, WeDiag1_t, DH
                xrT_dr, xr_nm_l, We_rep, inva_rep, bias_rep = xrT_dram, xr_nm, \
                    We1r_t, inva1_t, bias1_t
            else:
                D, p1, table, WeD_t, DIN2 = DO, P2, table2, WeDiag2_t, DO
                xrT_dr, xr_nm_l, We_rep, inva_rep, bias_rep = xr2T_dram, \
                    xr2_nm, We2r_t, inva2_t, bias2_t

            def do_block(j, pv, sb, sm):
                Kj, KAj, KBj = int(K[j]), int(KA[j]), int(KB[j])
                W = Kj * DH
                vtag = "vb" if Kj > VSMALL else "v"
                ps_v = pv.tile([P, W], f32, tag=vtag, space="PSUM")
                g_t = sb.tile([P, Kj, DH], f32, tag="g")
                nc.gpsimd.dma_gather(
                    g_t[:, 0:KAj, :], table[0:min(SPLIT, N), :],
                    idxA_t[:, int(colsA[j]):int(colsA[j + 1])],
                    P * KAj, P * KAj, DH, single_packet=False)
                if KBj:
                    nc.gpsimd.dma_gather(
                        g_t[:, KAj:Kj, :], table[SPLIT:N, :],
                        idxB_t[:, int(colsB[j]):int(colsB[j + 1])],
                        P * KBj, P * KBj, DH, single_packet=False)
                wTt = sb.tile([KMAX, P], f32, tag="wtt")
                nc.scalar.dma_start(wTt[:], wT_d[:, j * P:(j + 1) * P])
                xrb = xr_nm_l[:, j * DH:(j + 1) * DH]
                g_flat = g_t[:].rearrange("p k d -> p (k d)")
                for g0 in range(0, W, 512):
                    g1 = min(g0 + 512, W)
                    last = g1 == W
                    nc.tensor.matmul(out=ps_v[:, g0:g1], lhsT=wTt[0:Kj, :],
                                     rhs=WeD_t[0:Kj, g0:g1],
                                     start=True, stop=False)
                    nc.tensor.matmul(
                        out=ps_v[:, g0:g1], lhsT=ident[:],
                        rhs=bass.AP(xrb.tensor, xrb.offset,
                                    [xrb.ap[0], [0, (g1 - g0) // DH],
                                     xrb.ap[1]]),
                        start=False, stop=False)
                    nc.tensor.matmul(out=ps_v[:, g0:g1], lhsT=ident[:],
                                     rhs=g_flat[:, g0:g1],
                                     start=False, stop=True)
                if DBG and layer == 1 and j == 0:
                    nc.sync.dma_start(out=dbg["dbg_g"][:],
                                      in_=g_flat[:])
                m_t = sb.tile([P, W], f32, tag="m")
                nc.scalar.activation(out=m_t[:], in_=ps_v[:], func=Act.Prelu,
                                     alpha=0.2)
                v_sb = sb.tile([P, W], f32, tag="vs")
                nc.scalar.activation(out=v_sb[:], in_=ps_v[:],
                                     func=Act.Identity, bias=0.0)
                if DBG and layer == 1 and j == 0:
                    vd_t = sb.tile([P, W], f32, tag="vd")
                    nc.vector.tensor_copy(out=vd_t[:], in_=ps_v[:])
                    nc.sync.dma_start(out=dbg["dbg_v"][:], in_=vd_t[:])
                    nc.sync.dma_start(out=dbg["dbg_m"][:], in_=m_t[:])
                m3d = m_t[:].rearrange("p (k d) -> p k d", d=DH)
                sp_t = sm.tile([P, Kj], f32, tag="sp")
                sn_t = sm.tile([P, Kj], f32, tag="sn")
                s_t = sm.tile([P, Kj], f32, tag="s")
                if 0 < p1:
                    nc.vector.tensor_reduce(out=sp_t[:], in_=m3d[:, :, 0:p1],
                                            axis=mybir.AxisListType.X,
                                            op=Op.add)
                if p1 < D:
                    nc.vector.tensor_reduce(out=sn_t[:], in_=m3d[:, :, p1:D],
                                            axis=mybir.AxisListType.X,
                                            op=Op.add)
                if 0 < p1 < D:
                    nc.vector.scalar_tensor_tensor(
                        out=s_t[:], in0=sn_t[:], scalar=-1.0, in1=sp_t[:],
                        op0=Op.mult, op1=Op.add)
                elif p1 == D:
                    nc.vector.tensor_copy(out=s_t[:], in_=sp_t[:])
                else:
                    nc.vector.tensor_scalar(out=s_t[:], in0=sn_t[:],
                                            scalar1=-1.0, scalar2=None,
                                            op0=Op.mult)
                e_t = sm.tile([P, Kj], f32, tag="e")
                nc.scalar.activation(out=e_t[:], in_=s_t[:], func=Act.Exp)
                e2_t = sm.tile([P, Kj], f32, tag="e2")
                nc.vector.tensor_tensor(
                    out=e2_t[:], in0=e_t[:],
                    in1=m01_t[:, int(koff[j]):int(koff[j]) + Kj], op=Op.mult)
                Z_t = sm.tile([P, 1], f32, tag="Z")
                nc.vector.tensor_reduce(out=Z_t[:], in_=e2_t[:],
                                        axis=mybir.AxisListType.X, op=Op.add)
                iZ_t = sm.tile([P, 1], f32, tag="iZ")
                nc.vector.reciprocal(out=iZ_t[:], in_=Z_t[:])
                al_t = sm.tile([P, Kj], f32, tag="al")
                nc.vector.tensor_scalar(out=al_t[:], in0=e2_t[:],
                                        scalar1=iZ_t[:], scalar2=None,
                                        op0=Op.mult)
                aw_t = sm.tile([P, Kj], f32, tag="aw")
                zw_t = sm.tile([P, 1], f32, tag="zw")
                nc.vector.scalar_tensor_tensor(
                    out=aw_t[:], in0=al_t[:], scalar=1.0,
                    in1=w_t[:, int(koff[j]):int(koff[j]) + Kj],
                    op0=Op.mult, op1=Op.mult, accum_out=zw_t[:])
                nzw_t = sm.tile([P, 1], f32, tag="nzw")
                nc.vector.tensor_scalar(out=nzw_t[:], in0=zw_t[:],
                                        scalar1=-1.0, scalar2=None, op0=Op.mult)
                if DBG and layer == 1 and j == 0:
                    nc.sync.dma_start(out=dbg["dbg_s"][:], in_=s_t[:])
                    nc.sync.dma_start(out=dbg["dbg_e2"][:], in_=e2_t[:])
                    nc.sync.dma_start(out=dbg["dbg_al"][:], in_=al_t[:])
                nc.vector.tensor_tensor(out=m_t[:], in0=v_sb[:],
                                        in1=bcast_inner(al_t[:], DH),
                                        op=Op.mult)
                agg_t = sm.tile([P, DH], f32, tag="agg")
                nc.vector.tensor_reduce(
                    out=agg_t[:],
                    in_=m_t[:].rearrange("p (k d) -> p d k", d=DH),
                    axis=mybir.AxisListType.X, op=Op.add)
                t2_t = sm.tile([P, D], f32, tag="t2")
                nc.vector.scalar_tensor_tensor(
                    out=t2_t[:], in0=xr_nm_l[:, j * DH:j * DH + D], scalar=-1.0,
                    in1=agg_t[:, :D], op0=Op.mult, op1=Op.add)
                t3_t = sm.tile([P, D], f32, tag="t3")
                nc.vector.scalar_tensor_tensor(
                    out=t3_t[:], in0=We_rep[:], scalar=nzw_t[:], in1=t2_t[:],
                    op0=Op.mult, op1=Op.add)
                t4_t = sm.tile([P, D], f32, tag="t4")
                nc.vector.tensor_tensor(out=t4_t[:], in0=t3_t[:],
                                        in1=inva_rep[:], op=Op.mult)
                t5_t = sm.tile([P, D], f32, tag="t5")
                nc.vector.tensor_tensor(out=t5_t[:], in0=t4_t[:],
                                        in1=bias_rep[:], op=Op.add)
                if DBG and layer == 1 and j == 0:
                    nc.sync.dma_start(out=dbg["dbg_agg"][:], in_=agg_t[:])
                    nc.sync.dma_start(out=dbg["dbg_t5"][:], in_=t5_t[:])
                if layer == 1:
                    u1_t = sm.tile([P, D], f32, tag="u1")
                    nc.vector.tensor_scalar(out=u1_t[:], in0=t5_t[:],
                                            scalar1=0.0, scalar2=None,
                                            op0=Op.min)
                    u2_t = sm.tile([P, D], f32, tag="u2")
                    nc.scalar.activation(out=u2_t[:], in_=u1_t[:], func=Act.Exp)
                    nc.vector.scalar_tensor_tensor(
                        out=h_t[:, j * DH:(j + 1) * DH], in0=t5_t[:],
                        scalar=0.0, in1=u2_t[:], op0=Op.max, op1=Op.add)
                    if DBG and j == 0:
                        nc.sync.dma_start(out=dbg["dbg_h"][:],
                                          in_=h_t[:, 0:DH])
                else:
                    # softplus(z) = relu(z) + ln(1 + exp(-|z|))
                    ab_t = sm.tile([P, D], f32, tag="ab")
                    nc.scalar.activation(out=ab_t[:], in_=t5_t[:], func=Act.Abs)
                    ex_t = sm.tile([P, D], f32, tag="ex")
                    nc.scalar.activation(out=ex_t[:], in_=ab_t[:], func=Act.Exp,
                                         scale=-1.0)
                    ln_t = sm.tile([P, D], f32, tag="ln")
                    nc.scalar.activation(out=ln_t[:], in_=ex_t[:], func=Act.Ln,
                                         bias=1.0)
                    sp2_t = sm.tile([P, D], f32, tag="sp2")
                    nc.vector.scalar_tensor_tensor(
                        out=sp2_t[:], in0=t5_t[:], scalar=0.0, in1=ln_t[:],
                        op0=Op.max, op1=Op.add)
                    o_t = sm.tile([P, D], f32, tag="o")
                    nc.vector.tensor_scalar(out=o_t[:], in0=sp2_t[:],
                                            scalar1=1e-4, scalar2=None,
                                            op0=Op.add)
                    nc.sync.dma_start(out=out_d[j * P:(j + 1) * P, :],
                                      in_=o_t[:])

            big = [j for j in range(NB) if K[j] > VSMALL]
            small = [j for j in range(NB) if K[j] <= VSMALL]
            with (
                tc.tile_pool(name=f"pvb{layer}", bufs=1, space="PSUM") as pvb,
                tc.tile_pool(name=f"sbb{layer}", bufs=2) as sbb,
                tc.tile_pool(name=f"smb{layer}", bufs=2) as smb,
            ):
                for j in big:
                    do_block(j, pvb, sbb, smb)
            with (
                tc.tile_pool(name=f"pv{layer}", bufs=2, space="PSUM") as pv,
                tc.tile_pool(name=f"sb{layer}", bufs=3) as sb,
                tc.tile_pool(name=f"sm{layer}", bufs=3) as sm,
            ):
                for j in small:
                    do_block(j, pv, sb, sm)

        # ---------------------------- schedule ---------------------------
        with tc.tile_pool(name="xt", bufs=1) as xtp:
            xT_s = xtp.tile([D_IN, NLP], f32)
            nc.sync.dma_start(xT_s[:], xT_d[:])
            dense(DH, xT_s[:], Wl1T_t[:], bl1_t[:], Wr1T_t[:], br1_t[:],
                  xrT_dram, xr_nm, bounce1, pad=False)

        nc.gpsimd.collective_compute(
            "AllGather", Op.bypass, replica_groups=[list(range(C))],
            ins=[bounce1[:]], outs=[table1[:]])

        if STAGE == 1:  # dump a slice of table1 into out[0:P] and stop
            with tc.tile_pool(name="dbgp", bufs=2) as dp:
                tt = dp.tile([P, DO], f32)
                nc.sync.dma_start(tt[:], table1[0:P, 0:DO])
                nc.sync.dma_start(out_d[0:P, :], tt[:])
        elif STAGE == 2:  # one gather from table1, dump slot 0
            with tc.tile_pool(name="dbgp", bufs=2) as dp:
                Kj, KAj = int(K[0]), int(KA[0])
                gt = dp.tile([P, Kj, DH], f32)
                nc.gpsimd.dma_gather(
                    gt[:, 0:KAj, :], table1[0:min(SPLIT, N), :],
                    idxA_t[:, 0:int(colsA[1])], P * KAj, P * KAj, DH,
                    single_packet=False)
                ot = dp.tile([P, DO], f32)
                nc.vector.tensor_copy(out=ot[:], in_=gt[:, 0, 0:DO])
                nc.sync.dma_start(out_d[0:P, :], ot[:])
        if STAGE not in (1, 2):
            edge(1)

            if STAGE == 3:
                with tc.tile_pool(name="dbgp", bufs=2) as dp:
                    ot = dp.tile([P, DO], f32)
                    nc.vector.tensor_copy(out=ot[:], in_=h_t[:, 0:DO])
                    nc.sync.dma_start(out_d[0:P, :], ot[:])
            else:
                with tc.tile_pool(name="hT", bufs=1) as hTp:
                    hT_t = hTp.tile([DH, NLP], f32)
                    with tc.tile_pool(name="htr", bufs=2, space="PSUM") as htr:
                        for j in range(NB):
                            ps = htr.tile([DH, P], f32, tag="h", space="PSUM")
                            nc.tensor.transpose(
                                out=ps[:], in_=h_t[:, j * DH:(j + 1) * DH],
                                identity=ident[:])
                            nc.scalar.activation(
                                out=hT_t[:, j * P:(j + 1) * P],
                                in_=ps[:], func=Act.Identity, bias=0.0)
                    dense(DO, hT_t[:], Wl2T_t[:], bl2_t[:], Wr2T_t[:],
                          br2_t[:], xr2T_dram, xr2_nm, bounce2, pad=True)

                nc.gpsimd.collective_compute(
                    "AllGather", Op.bypass, replica_groups=[list(range(C))],
                    ins=[bounce2[:]], outs=[table2[:]])

                edge(2)

    nc.compile()
    return nc


# ----------------------------------------------------------------------------
# entry point
# ----------------------------------------------------------------------------

def _make_in_maps(inputs):
    x = np.asarray(inputs["x"], np.float32)
    f1 = _fold(inputs["Wl1"], inputs["bl1"], inputs["Wr1"], inputs["br1"],
               inputs["We1"], inputs["att1"], inputs["bias1"])
    f2 = _fold(inputs["Wl2"], inputs["bl2"], inputs["Wr2"], inputs["br2"],
               inputs["We2"], inputs["att2"], inputs["bias2"],
               in_perm=f1["pi"], h_offset=True)
    g = _prep(x, inputs["edge_index"], inputs["edge_weight"])
    WeDiag1, IdTile1, WeDiag2, IdTile2 = _consts(g["KMAX"], f1["We"], f2["We"])

    plan = dict(g, p1=f1["p1"], p2=f2["p1"])
    shared = dict(
        Wl1T=f1["WlT"], Wr1T=f1["WrT"], bl1=f1["bl"], br1=f1["br"],
        Wl2T=f2["WlT"], Wr2T=f2["WrT"], bl2=f2["bl"], br2=f2["br"],
        WeDiag1=WeDiag1, IdTile1=IdTile1, WeDiag2=WeDiag2, IdTile2=IdTile2,
        We1r=f1["We"][None, :], inva1=f1["inva"][None, :],
        bias1=f1["bias"][None, :],
        We2r=f2["We"][None, :], inva2=f2["inva"][None, :],
        bias2=f2["bias"][None, :],
    )
    in_maps = []
    for c in range(C):
        m = dict(shared)
        m.update(xT=g["xT"][c], idxA=g["idxA"][c], idxB=g["idxB"][c],
                 w_arr=g["w_arr"][c], m01=g["m01"][c], wT=g["wT"][c])
        in_maps.append(m)
    return plan, in_maps, g, f2


def kernel(**inputs):
    from concourse.bass_utils import run_bass_kernel_spmd

    plan, in_maps, g, f2 = _make_in_maps(inputs)
    nc = _build(plan)
    res = run_bass_kernel_spmd(nc, in_maps, list(range(C)))

    full_new = np.concatenate([res.results[c]["out"][:NL] for c in range(C)], 0)
    full_old = full_new[g["new_id"]]
    out = np.empty((N, DO), np.float32)
    out[:, f2["pi"]] = full_old
    return out.astype(np.float32)



# revision 9
# speedup vs baseline: 1.4048x; 1.4048x over previous
"""Two-layer GATv2 (heads=1, edge_dim=1) on 8 Trainium2 NeuronCores.

Sharding: nodes dealt round-robin by in-degree onto 8 cores; dst-grouped
edges stay local; source features come from an AllGather'd full table via
dma_gather on three concurrent SWDGE queues (queues 1-3 dispatch in ~0.4us
and generate descriptors on their own Q7 core pairs in parallel).

Per 128-node (dst) block:
  v   = [xrT | w]-lhsT @ [IdTile | WeDiag] + ident @ g    -- 2 matmuls/chunk
  m   = prelu(v, 0.2)                                     -- ACT (from PSUM)
  s   = sum_{d<P1} m - sum_{d>=P1} m                      -- att sign-split
  e   = exp(s), Z = row-sum(e)                            -- ACT with accum
  agg = sum_k e_k * g_k                                   -- DVE mult+reduce
  out = agg * inva * (1/Z) + bias                         -- undo |att| scale
Padded gather slots point at a per-core "fake" table row holding -/+1e6 in
the sign-sorted layout, so exp(s)==0 exactly -- no mask tensors needed.
Dense phase computes xl node-major (lhsT = xT block) and xr feature-major
(lhsT = WrT) directly -- no PE transposes; biases fold into rank-1 matmul /
ACT bias. Layer-2 softplus runs on ACT; the +1e-4 is added host-side.
"""

import numpy as np

N, E, D_IN, DH, DO = 50000, 800000, 128, 64, 32
C = 8                      # cores
NL = N // C                # nodes per core (6250)
P = 128                    # partitions = nodes per block
NB = (NL + P - 1) // P     # blocks per core (49)
NLP = NB * P               # padded nodes per core (6272)
NLF = 6256                 # bounce rows per core (6250 real + fake + pad)
SPLIT = 32768              # int16 gather table split (table-row space)
NT = C * NLF               # table rows (50048)
VSMALL = 32                # K threshold for double-buffered PSUM v-tiles
FAKE_A = 6250              # core-0 fake row (side A)
FAKE_B = 5 * NLF + 6250 - SPLIT   # core-5 fake row offset in side B
GQ = (1, 2, 3)             # SWDGE queues for gathers (0 blocks the sequencer)


# ----------------------------------------------------------------------------
# host-side: weight folding and graph layout
# ----------------------------------------------------------------------------

def _fold(Wl, bl, Wr, br, We, att, bias, in_perm=None, h_offset=False):
    att = np.asarray(att, np.float64)
    pi = np.concatenate([np.nonzero(att >= 0)[0], np.nonzero(att < 0)[0]])
    p1 = int((att >= 0).sum())
    a = np.maximum(np.abs(att[pi]), 1e-30)
    Wl = np.asarray(Wl, np.float64)[pi] * a[:, None]
    Wr = np.asarray(Wr, np.float64)[pi] * a[:, None]
    bl = np.asarray(bl, np.float64)[pi] * a
    br = np.asarray(br, np.float64)[pi] * a
    We_ = np.asarray(We, np.float64)[pi, 0] * a
    if in_perm is not None:
        Wl = Wl[:, in_perm]
        Wr = Wr[:, in_perm]
    if h_offset:  # input arrives as h+1
        bl = bl - Wl.sum(1)
        br = br - Wr.sum(1)
    return dict(
        WlT=np.ascontiguousarray(Wl.T, np.float32),
        WrT=np.ascontiguousarray(Wr.T, np.float32),
        blRow=bl.astype(np.float32)[None, :],
        br=br.astype(np.float32)[:, None],
        brRow=br.astype(np.float32)[None, :],
        We=We_.astype(np.float32),
        inva=(1.0 / a).astype(np.float32),
        bias=np.asarray(bias, np.float64)[pi].astype(np.float32),
        pi=pi, p1=p1,
    )


def _prep(x, edge_index, edge_weight):
    src = np.asarray(edge_index[0], np.int64)
    dst = np.asarray(edge_index[1], np.int64)
    w = np.asarray(edge_weight, np.float32)

    deg = np.bincount(dst, minlength=N)
    wsum = np.bincount(dst, weights=w.astype(np.float64), minlength=N)
    loop_w = (wsum / np.maximum(deg, 1)).astype(np.float32)

    order = np.argsort(-deg, kind="stable")
    ranks = np.empty(N, np.int64)
    ranks[order] = np.arange(N)
    core = ranks % C
    crank = ranks // C
    new_id = core * NL + crank          # output-row space
    trow = core * NLF + crank           # gather-table-row space
    inv = np.empty(N, np.int64)
    inv[new_id] = np.arange(N)          # old id of each new id

    esrc = np.concatenate([trow[src], trow])          # table rows of sources
    edst = np.concatenate([new_id[dst], new_id])      # local ids of dsts
    ew = np.concatenate([w, loop_w]).astype(np.float32)

    side = (esrc >= SPLIT).astype(np.int64)
    eord = np.argsort(edst * 2 + side, kind="stable")
    sdst, ssrc, sw, sside = edst[eord], esrc[eord], ew[eord], side[eord]

    nA = np.bincount(edst[side == 0], minlength=N)
    nB = np.bincount(edst[side == 1], minlength=N)

    grp = np.searchsorted(sdst, np.arange(N))
    pos = np.arange(E + N) - grp[sdst]
    posAB = np.where(sside == 0, pos, pos - nA[sdst])

    nblk = (np.arange(N) % NL) // P      # block of each new id
    KA = np.zeros(NB, np.int64)
    KB = np.zeros(NB, np.int64)
    np.maximum.at(KA, nblk, nA)
    np.maximum.at(KB, nblk, nB)
    KA = np.maximum(KA, 1)
    K = KA + KB
    assert int(K.max()) * DH * 4 <= 16384, f"KMAX {K.max()} overflows PSUM"
    KMAX = int(K.max())
    totK = int(K.sum())

    e_core = sdst // NL
    e_loc = sdst % NL
    e_blk = e_loc // P
    e_p = e_loc % P
    e_k = np.where(sside == 0, posAB, KA[e_blk] + posAB)

    colsA = np.concatenate([[0], np.cumsum(KA * 8)]).astype(np.int64)
    colsB = np.concatenate([[0], np.cumsum(KB * 8)]).astype(np.int64)
    idxA = np.full((C, 128, int(colsA[-1])), FAKE_A, np.int16)
    idxB = np.full((C, 128, max(int(colsB[-1]), 16)), FAKE_B, np.int16)
    mA = sside == 0
    fA = e_k[mA] * P + e_p[mA]
    idxA[e_core[mA], fA % 16, colsA[e_blk[mA]] + fA // 16] = \
        ssrc[mA].astype(np.int16)
    mB = ~mA
    fB = (e_k[mB] - KA[e_blk[mB]]) * P + e_p[mB]
    idxB[e_core[mB], fB % 16, colsB[e_blk[mB]] + fB // 16] = \
        (ssrc[mB] - SPLIT).astype(np.int16)
    if NL % P:  # dummy partitions in last block: avoid Z=0 (point at row 0)
        for p in range(NL % P, P):
            f = 0 * P + p
            idxA[:, f % 16, colsA[NB - 1] + f // 16] = 0
    for rep in range(1, 8):
        idxA[:, 16 * rep:16 * rep + 16] = idxA[:, :16]
        idxB[:, 16 * rep:16 * rep + 16] = idxB[:, :16]

    x = np.asarray(x, np.float32)
    xT = np.zeros((C, D_IN, NLP), np.float32)
    perm = inv.reshape(C, NL)
    for c in range(C):
        xT[c, :, :NL] = x[perm[c]].T

    wT = np.zeros((C, KMAX, NLP), np.float32)
    wT[e_core, e_k, e_blk * P + e_p] = sw

    return dict(new_id=new_id, K=K, KA=KA, KB=KB, KMAX=KMAX, totK=totK,
                colsA=colsA, colsB=colsB, idxA=idxA, idxB=idxB, xT=xT, wT=wT)


def _consts(KMAX, We1, We2, p1, p2):
    # layer 1: rhsCat1 = [IdTile1 (64 rows); WeDiag1 (KMAX rows)]
    H1 = DH + KMAX
    rhs1 = np.zeros((H1, KMAX * DH), np.float32)
    for d in range(DH):
        rhs1[d, d::DH] = 1.0
    for k in range(KMAX):
        rhs1[DH + k, k * DH:(k + 1) * DH] = We1
    # layer 2: rhsCat2 = [IdTile2 (32 rows); WeDiag2 (KMAX rows)], 64-col slots
    H2 = DO + KMAX
    rhs2 = np.zeros((H2, KMAX * DH), np.float32)
    for d in range(DO):
        rhs2[d, d::DH] = 1.0
    for k in range(KMAX):
        rhs2[DO + k, k * DH:k * DH + DO] = We2
    fake1 = np.where(np.arange(DH) < p1, -1e6, 1e6).astype(np.float32)[None, :]
    f2 = np.zeros(DH, np.float32)
    f2[:DO] = np.where(np.arange(DO) < p2, -1e6, 1e6)
    fake2 = f2[None, :]
    return rhs1, rhs2, fake1, fake2


# ----------------------------------------------------------------------------
# device program
# ----------------------------------------------------------------------------

def _build(plan):
    import concourse.bacc as bacc
    import concourse.bass as bass
    import concourse.mybir as mybir
    import concourse.tile as tile
    from concourse.library_config import mlp
    from concourse.masks import make_identity

    f32 = mybir.dt.float32
    i16 = mybir.dt.int16
    Op = mybir.AluOpType
    Act = mybir.ActivationFunctionType

    K, KA, KB = plan["K"], plan["KA"], plan["KB"]
    KMAX, totK = plan["KMAX"], plan["totK"]
    colsA, colsB = plan["colsA"], plan["colsB"]
    P1, P2 = plan["p1"], plan["p2"]
    nA_cols = int(colsA[-1])
    nB_cols = max(int(colsB[-1]), 16)
    H1, H2 = DH + KMAX, DO + KMAX

    nc = bacc.Bacc("TRN2", debug=False, num_swdge_queues=4)

    def din(name, shape, dt=f32):
        return nc.dram_tensor(name, shape, dt, kind="ExternalInput")

    xT_d = din("xT", [D_IN, NLP])
    idxA_d = din("idxA", [128, nA_cols], i16)
    idxB_d = din("idxB", [128, nB_cols], i16)
    wT_d = din("wT", [KMAX, NLP])
    rhs1_d = din("rhs1", [H1, KMAX * DH])
    rhs2_d = din("rhs2", [H2, KMAX * DH])
    Wl1T_d, Wr1T_d = din("Wl1T", [D_IN, DH]), din("Wr1T", [D_IN, DH])
    bl1R_d, br1_d = din("bl1R", [1, DH]), din("br1", [DH, 1])
    Wl2b_d = din("Wl2b", [DH + 1, DO])   # [Wl2T; bl2 row]
    Wr2b_d = din("Wr2b", [DH + 1, DO])   # [Wr2T; br2 row]
    inva1_d, bias1_d = din("inva1", [1, DH]), din("bias1", [1, DH])
    inva2_d, bias2_d = din("inva2", [1, DO]), din("bias2", [1, DO])
    fake1_d = din("fake1", [1, DH])
    fake2_d = din("fake2", [1, DH])

    out_d = nc.dram_tensor("out", [NLP, DO], f32, kind="ExternalOutput")

    bounce1 = nc.dram_tensor("bounce1", [NLF, DH], f32)
    table1 = nc.dram_tensor("table1", [NT, DH], f32)
    bounce2 = nc.dram_tensor("bounce2", [NLF, DH], f32)
    table2 = nc.dram_tensor("table2", [NT, DH], f32)

    with tile.TileContext(nc) as tc:
      with tc.tile_pool(name="persist", bufs=1) as pp:
        ident = pp.tile([P, P], f32)
        make_identity(nc, ident[:])
        nc.gpsimd.load_library(mlp)

        idxA_t = pp.tile([128, nA_cols], i16)
        idxB_t = pp.tile([128, nB_cols], i16)
        rhs1_t = pp.tile([H1, KMAX * DH], f32)
        rhs2_t = pp.tile([H2, KMAX * DH], f32)
        LT1 = pp.tile([H1, NLP], f32)     # rows 0:64 xr1T, 64:H1 wT
        LT2 = pp.tile([H2, NLP], f32)     # rows 0:32 xr2T, 32:H2 wT
        hT = pp.tile([DH + 1, NLP], f32)  # row 64 = ones (bias fold)
        Wl1T_t = pp.tile([D_IN, DH], f32, tag="Wl1T_t")
        Wr1T_t = pp.tile([D_IN, DH], f32, tag="Wr1T_t")
        bl1R_t = pp.tile([1, DH], f32, tag="bl1R_t")
        br1_t = pp.tile([DH, 1], f32, tag="br1_t")
        Wl2b_t = pp.tile([DH + 1, DO], f32, tag="Wl2b_t")
        Wr2b_t = pp.tile([DH + 1, DO], f32, tag="Wr2b_t")
        ones1_t = pp.tile([1, P], f32, tag="ones1_t")
        inva1_t = pp.tile([P, DH], f32)
        bias1_t = pp.tile([P, DH], f32)
        inva2_t = pp.tile([P, DO], f32)
        bias2_t = pp.tile([P, DO], f32)
        fake1_t = pp.tile([1, DH], f32, tag="fake1_t")
        fake2_t = pp.tile([1, DH], f32, tag="fake2_t")

        nc.vector.memset(hT[DH:DH + 1, :], 1.0)
        nc.vector.memset(ones1_t[:], 1.0)

        for t, d in [(idxA_t, idxA_d), (idxB_t, idxB_d),
                     (rhs1_t, rhs1_d), (rhs2_t, rhs2_d),
                     (Wl1T_t, Wl1T_d), (Wr1T_t, Wr1T_d),
                     (bl1R_t, bl1R_d), (br1_t, br1_d),
                     (Wl2b_t, Wl2b_d), (Wr2b_t, Wr2b_d),
                     (fake1_t, fake1_d), (fake2_t, fake2_d)]:
            nc.sync.dma_start(t[:], d[:])
        nc.sync.dma_start(LT1[DH:DH + KMAX, :], wT_d[:])
        nc.sync.dma_start(LT2[DO:DO + KMAX, :], wT_d[:])
        for t, d, dd in [(inva1_t, inva1_d, DH), (bias1_t, bias1_d, DH),
                         (inva2_t, inva2_d, DO), (bias2_t, bias2_d, DO)]:
            nc.sync.dma_start(t[:], d[:].to_broadcast([P, dd]))
        nc.sync.dma_start(bounce1[6250:6251, :], fake1_t[:])
        nc.sync.dma_start(bounce2[6250:6251, :], fake2_t[:])

        def bcast_inner(ap, n):
            return bass.AP(ap.tensor, ap.offset, [*ap.ap, [0, n]])

        # ------------------ dense phase (either layer) -------------------
        def dense(layer):
            """xl node-major -> bounce rows; xr feature-major -> LT rows."""
            if layer == 1:
                DOUT, LT, bounce = DH, LT1, bounce1
            else:
                DOUT, LT, bounce = DO, LT2, bounce2
            with (
                tc.tile_pool(name=f"dps{layer}", bufs=2, space="PSUM") as dps,
                tc.tile_pool(name=f"dsb{layer}", bufs=3) as dsb,
            ):
                for j in range(NB):
                    c0 = j * P
                    if layer == 1:
                        lhs_blk = xT_s[:, c0:c0 + P]
                        ps = dps.tile([P, DOUT], f32, tag="xl", space="PSUM")
                        nc.tensor.matmul(out=ps[:], lhsT=lhs_blk,
                                         rhs=Wl1T_t[:], start=True, stop=False)
                        nc.tensor.matmul(out=ps[:], lhsT=ones1_t[:],
                                         rhs=bl1R_t[:], start=False, stop=True,
                                         skip_group_check=True)
                        ps2 = dps.tile([DOUT, P], f32, tag="xr", space="PSUM")
                        nc.tensor.matmul(out=ps2[:], lhsT=Wr1T_t[:],
                                         rhs=lhs_blk, start=True, stop=True)
                        nc.scalar.activation(out=LT[0:DOUT, c0:c0 + P],
                                             in_=ps2[:], func=Act.Identity,
                                             bias=br1_t[:])
                    else:
                        lhs_blk = hT[:, c0:c0 + P]
                        ps = dps.tile([P, DOUT], f32, tag="xl", space="PSUM")
                        nc.tensor.matmul(out=ps[:], lhsT=lhs_blk,
                                         rhs=Wl2b_t[:], start=True, stop=True)
                        ps2 = dps.tile([DOUT, P], f32, tag="xr", space="PSUM")
                        nc.tensor.matmul(out=ps2[:], lhsT=Wr2b_t[:],
                                         rhs=lhs_blk, start=True, stop=True)
                        nc.scalar.activation(out=LT[0:DOUT, c0:c0 + P],
                                             in_=ps2[:], func=Act.Identity,
                                             bias=0.0)
                    st = dsb.tile([P, DH], f32, tag="st")
                    if layer == 2:
                        nc.vector.memset(st[:], 0.0)
                    nc.scalar.activation(out=st[:, 0:DOUT], in_=ps[:],
                                         func=Act.Identity, bias=0.0)
                    lo, hi = j * P, min((j + 1) * P, NL)
                    if hi > lo:
                        nc.sync.dma_start(out=bounce[lo:hi, :],
                                          in_=st[:hi - lo, :])

        # ------------------ edge phase (either layer) --------------------
        qctr = [0]

        def edge(layer):
            if layer == 1:
                D, p1, table, LT, rhs_t, H = DH, P1, table1, LT1, rhs1_t, H1
                inva_rep, bias_rep = inva1_t, bias1_t
            else:
                D, p1, table, LT, rhs_t, H = DO, P2, table2, LT2, rhs2_t, H2
                inva_rep, bias_rep = inva2_t, bias2_t

            def do_block(j, pv, sb, sm, ph):
                Kj, KAj, KBj = int(K[j]), int(KA[j]), int(KB[j])
                W = Kj * DH
                g_t = sb.tile([P, KMAX, DH], f32, tag="g")
                q = GQ[qctr[0] % len(GQ)]
                qctr[0] += 1
                nc.gpsimd.dma_gather(
                    g_t[:, 0:KAj, :], table[0:SPLIT, :],
                    idxA_t[:, int(colsA[j]):int(colsA[j + 1])],
                    P * KAj, P * KAj, DH, single_packet=False, queue_num=q)
                if KBj:
                    q = GQ[qctr[0] % len(GQ)]
                    qctr[0] += 1
                    nc.gpsimd.dma_gather(
                        g_t[:, KAj:Kj, :], table[SPLIT:NT, :],
                        idxB_t[:, int(colsB[j]):int(colsB[j + 1])],
                        P * KBj, P * KBj, DH, single_packet=False, queue_num=q)
                g_flat = g_t[:].rearrange("p k d -> p (k d)")
                lhs_blk = LT[:, j * P:(j + 1) * P]
                m_t = sb.tile([P, W], f32, tag="m")
                for g0 in range(0, W, 512):
                    g1 = min(g0 + 512, W)
                    ps_v = pv.tile([P, 512], f32, tag="v", space="PSUM")
                    nc.tensor.matmul(out=ps_v[:, 0:g1 - g0], lhsT=lhs_blk,
                                     rhs=rhs_t[:, g0:g1],
                                     start=True, stop=False)
                    nc.tensor.matmul(out=ps_v[:, 0:g1 - g0], lhsT=ident[:],
                                     rhs=g_flat[:, g0:g1],
                                     start=False, stop=True)
                    nc.scalar.activation(out=m_t[:, g0:g1],
                                         in_=ps_v[:, 0:g1 - g0],
                                         func=Act.Prelu, alpha=0.2)
                m3d = m_t[:].rearrange("p (k d) -> p k d", d=DH)
                sp_t = sm.tile([P, KMAX], f32, tag="sp")
                sn_t = sm.tile([P, KMAX], f32, tag="sn")
                s_t = sm.tile([P, KMAX], f32, tag="s")
                if 0 < p1:
                    nc.vector.tensor_reduce(out=sp_t[:, 0:Kj],
                                            in_=m3d[:, :, 0:p1],
                                            axis=mybir.AxisListType.X,
                                            op=Op.add)
                if p1 < D:
                    nc.vector.tensor_reduce(out=sn_t[:, 0:Kj],
                                            in_=m3d[:, :, p1:D],
                                            axis=mybir.AxisListType.X,
                                            op=Op.add)
                if 0 < p1 < D:
                    nc.vector.scalar_tensor_tensor(
                        out=s_t[:, 0:Kj], in0=sn_t[:, 0:Kj], scalar=-1.0,
                        in1=sp_t[:, 0:Kj], op0=Op.mult, op1=Op.add)
                elif p1 == D:
                    s_t = sp_t
                else:
                    nc.vector.tensor_scalar(out=s_t[:, 0:Kj],
                                            in0=sn_t[:, 0:Kj],
                                            scalar1=-1.0, scalar2=None,
                                            op0=Op.mult)
                e_t = sm.tile([P, KMAX], f32, tag="e")
                Z_t = sm.tile([P, 1], f32, tag="Z")
                nc.scalar.activation(out=e_t[:, 0:Kj], in_=s_t[:, 0:Kj],
                                     func=Act.Exp, accum_out=Z_t[:])
                iZ_t = sm.tile([P, 1], f32, tag="iZ")
                nc.vector.reciprocal(out=iZ_t[:], in_=Z_t[:])
                # m is dead after sp/sn: reuse its space for e*g (SBUF budget)
                nc.vector.tensor_tensor(out=m_t[:], in0=g_flat[:, 0:W],
                                        in1=bcast_inner(e_t[:, 0:Kj], DH),
                                        op=Op.mult)
                agg_t = sm.tile([P, DH], f32, tag="agg")
                nc.vector.tensor_reduce(
                    out=agg_t[:],
                    in_=m_t[:].rearrange("p (k d) -> p d k", d=DH),
                    axis=mybir.AxisListType.X, op=Op.add)
                t4_t = sm.tile([P, D], f32, tag="t4")
                nc.vector.tensor_tensor(out=t4_t[:], in0=agg_t[:, 0:D],
                                        in1=inva_rep[:], op=Op.mult)
                t5_t = sm.tile([P, D], f32, tag="t5")
                nc.vector.scalar_tensor_tensor(
                    out=t5_t[:], in0=t4_t[:], scalar=iZ_t[:],
                    in1=bias_rep[:], op0=Op.mult, op1=Op.add)
                if layer == 1:
                    u1_t = sm.tile([P, D], f32, tag="u1")
                    nc.vector.tensor_scalar(out=u1_t[:], in0=t5_t[:],
                                            scalar1=0.0, scalar2=None,
                                            op0=Op.min)
                    u2_t = sm.tile([P, D], f32, tag="u2")
                    nc.scalar.activation(out=u2_t[:], in_=u1_t[:],
                                         func=Act.Exp)
                    h_t = sm.tile([P, D], f32, tag="h")
                    nc.vector.scalar_tensor_tensor(
                        out=h_t[:], in0=t5_t[:], scalar=0.0, in1=u2_t[:],
                        op0=Op.max, op1=Op.add)
                    pst = ph.tile([DH, P], f32, tag="ht", space="PSUM")
                    nc.tensor.transpose(out=pst[:], in_=h_t[:],
                                        identity=ident[:])
                    nc.scalar.activation(out=hT[0:DH, j * P:(j + 1) * P],
                                         in_=pst[:], func=Act.Identity,
                                         bias=0.0)
                else:
                    # softplus(z) = relu(z) + ln(1 + exp(-|z|))
                    ab_t = sm.tile([P, D], f32, tag="ab")
                    nc.scalar.activation(out=ab_t[:], in_=t5_t[:],
                                         func=Act.Abs)
                    ex_t = sm.tile([P, D], f32, tag="ex")
                    nc.scalar.activation(out=ex_t[:], in_=ab_t[:],
                                         func=Act.Exp, scale=-1.0)
                    ln_t = sm.tile([P, D], f32, tag="ln")
                    nc.scalar.activation(out=ln_t[:], in_=ex_t[:],
                                         func=Act.Ln, bias=1.0)
                    o_t = sm.tile([P, D], f32, tag="o")
                    nc.vector.scalar_tensor_tensor(
                        out=o_t[:], in0=t5_t[:], scalar=0.0, in1=ln_t[:],
                        op0=Op.max, op1=Op.add)
                    nc.sync.dma_start(out=out_d[j * P:(j + 1) * P, :],
                                      in_=o_t[:])

            with (
                tc.tile_pool(name=f"pv{layer}", bufs=4, space="PSUM") as pv,
                tc.tile_pool(name=f"ph{layer}", bufs=2, space="PSUM") as ph,
                tc.tile_pool(name=f"sb{layer}", bufs=3) as sb,
                tc.tile_pool(name=f"sm{layer}", bufs=4) as sm,
            ):
                for j in range(NB):
                    do_block(j, pv, sb, sm, ph)

        # ---------------------------- schedule ---------------------------
        with tc.tile_pool(name="xt", bufs=1) as xtp:
            xT_s = xtp.tile([D_IN, NLP], f32)
            nc.sync.dma_start(xT_s[:], xT_d[:])
            dense(1)

        nc.gpsimd.collective_compute(
            "AllGather", Op.bypass, replica_groups=[list(range(C))],
            ins=[bounce1[:]], outs=[table1[:]])

        edge(1)
        dense(2)

        nc.gpsimd.collective_compute(
            "AllGather", Op.bypass, replica_groups=[list(range(C))],
            ins=[bounce2[:]], outs=[table2[:]])

        edge(2)

    nc.compile()
    return nc


# ----------------------------------------------------------------------------
# entry point
# ----------------------------------------------------------------------------

def _make_in_maps(inputs):
    x = np.asarray(inputs["x"], np.float32)
    f1 = _fold(inputs["Wl1"], inputs["bl1"], inputs["Wr1"], inputs["br1"],
               inputs["We1"], inputs["att1"], inputs["bias1"])
    f2 = _fold(inputs["Wl2"], inputs["bl2"], inputs["Wr2"], inputs["br2"],
               inputs["We2"], inputs["att2"], inputs["bias2"],
               in_perm=f1["pi"], h_offset=True)
    g = _prep(x, inputs["edge_index"], inputs["edge_weight"])
    rhs1, rhs2, fake1, fake2 = _consts(g["KMAX"], f1["We"], f2["We"],
                                       f1["p1"], f2["p1"])

    plan = dict(g, p1=f1["p1"], p2=f2["p1"])
    Wl2b = np.concatenate([f2["WlT"], f2["blRow"]], 0)
    Wr2b = np.concatenate([f2["WrT"], f2["brRow"]], 0)
    shared = dict(
        Wl1T=f1["WlT"], Wr1T=f1["WrT"], bl1R=f1["blRow"], br1=f1["br"],
        Wl2b=Wl2b, Wr2b=Wr2b,
        rhs1=rhs1, rhs2=rhs2, fake1=fake1, fake2=fake2,
        inva1=f1["inva"][None, :], bias1=f1["bias"][None, :],
        inva2=f2["inva"][None, :], bias2=f2["bias"][None, :],
    )
    in_maps = []
    for c in range(C):
        m = dict(shared)
        m.update(xT=g["xT"][c], idxA=g["idxA"][c], idxB=g["idxB"][c],
                 wT=g["wT"][c])
        in_maps.append(m)
    return plan, in_maps, g, f2


def kernel(**inputs):
    from concourse.bass_utils import run_bass_kernel_spmd

    plan, in_maps, g, f2 = _make_in_maps(inputs)
    nc = _build(plan)
    res = run_bass_kernel_spmd(nc, in_maps, list(range(C)))

    full_new = np.concatenate([res.results[c]["out"][:NL] for c in range(C)], 0)
    full_old = full_new[g["new_id"]]
    out = np.empty((N, DO), np.float32)
    out[:, f2["pi"]] = full_old + 1e-4
    return out.astype(np.float32)
